# revision 1
# baseline (speedup 1.0000x reference)
"""CRF loss kernel for Trainium2 (8 NeuronCores, data-parallel over batch).

Algorithm (per core, 64 sequences):
  Denominator (log-partition): exp-space forward recurrence
      s_t = (E^T s_{t-1}) * (1/82) (*) exp(logit_t),   E = exp(transitions)
  run as two half-batch groups skewed by one step so each DVE op advances
  both groups. Per step one PE matmul (blockdiag E plus an extra 2-column
  "exp(end)" readout matmul) and one DVE scalar_tensor_tensor. The per-step
  readout red_t[b] = sum_j s_t[j,b]*exp(end[j]) is logged (ACT Ln) so the
  per-sequence denominator is picked at t = L[b]-1 afterwards with a
  mask-derived one-hot contraction; the constant 1/82 rescale is undone on
  the host via (L-1)*ln(82).
  Numerator: per (b, t-chunk) PE matmuls against a host-packed
  [onehot(tag)*mask | scores | ohL | delta_t0] matrix give pair counts C,
  same-t products D, last-tag and first-tag one-hots in one PSUM tile;
  one tensor_tensor_reduce against [trans; I; end; start] sums everything.
"""

import os
import numpy as np
import ml_dtypes

import concourse.bass as bass
import concourse.bacc as bacc
import concourse.mybir as mybir
from concourse import tile
from concourse.bass_utils import run_bass_kernel_spmd

B, S, T = 512, 1024, 50
NCORES = 8
BL = B // NCORES  # 64
HB = BL // 2      # 32
P2 = 2 * T        # 100 (two stacked tag blocks)
CINV = np.float32(1.0 / 82.0)
LNC = float(np.log(np.float64(1.0) / np.float64(82.0)))
NEG = np.float32(-1e30)

WCH = 32          # windows per expl ring chunk
NWCH = (S + 1 + WCH - 1) // WCH + 1   # 33 chunks cover 1025 windows (pad to 1056)
NCH = 8           # numerator chunks (128 rows each)
CW = 166          # combo cols: ohm|scores|ohL|d0|ohmprev|zeros

TRACE = os.environ.get("CRF_TRACE") == "1"

_cached = {}


def _build_nc():
    f32 = mybir.dt.float32
    bf16 = mybir.dt.bfloat16
    AF = mybir.ActivationFunctionType
    OP = mybir.AluOpType

    nc = bacc.Bacc(None, target_bir_lowering=False)

    # ---- DRAM I/O ----
    d_sct2 = nc.dram_tensor("sct2", [P2, WCH * NWCH, HB], f32, kind="ExternalInput")
    d_combo = nc.dram_tensor("combo", [2, NCH, 128, HB, CW], bf16, kind="ExternalInput")
    d_ehatlog = nc.dram_tensor("ehatlog", [P2, 102], f32, kind="ExternalInput")
    d_startcol = nc.dram_tensor("startcol", [P2, 1], f32, kind="ExternalInput")
    d_cin1 = nc.dram_tensor("cin1", [116, T], f32, kind="ExternalInput")
    d_ident = nc.dram_tensor("ident", [128, 128], f32, kind="ExternalInput")
    d_maskip = nc.dram_tensor("maskip", [HB, 2052], f32, kind="ExternalInput")
    d_ohsel = nc.dram_tensor("ohsel", [128, 514], f32, kind="ExternalInput")
    d_sel = nc.dram_tensor("sel128", [128, HB], f32, kind="ExternalInput")
    d_ones = nc.dram_tensor("ones102", [116, 1], f32, kind="ExternalInput")

    d_num = nc.dram_tensor("o_num", [BL, 1], f32, kind="ExternalOutput")
    d_den = nc.dram_tensor("o_den", [HB, 2], f32, kind="ExternalOutput")
    d_li = nc.dram_tensor("o_li", [HB, 2], f32, kind="ExternalOutput")

    with tile.TileContext(nc) as tc:
        with (
            tc.tile_pool(name="const", bufs=1) as cpool,
            tc.tile_pool(name="ring", bufs=4) as ring,
            tc.tile_pool(name="state", bufs=3) as spool,
            tc.tile_pool(name="work", bufs=2) as wpool,
            tc.tile_pool(name="ps_state", bufs=2, space="PSUM") as ps_state,
            tc.tile_pool(name="ps_red", bufs=2, space="PSUM") as ps_red,
            tc.tile_pool(name="ps_cd", bufs=2, space="PSUM") as ps_cd,
            tc.tile_pool(name="ps_misc", bufs=2, space="PSUM") as ps_misc,
        ):
            # ---- constants ----
            ehatlog = cpool.tile([P2, 102], f32)
            nc.sync.dma_start(ehatlog[:], d_ehatlog[:])
            ehat = cpool.tile([P2, 102], f32)
            nc.scalar.activation(ehat[:], ehatlog[:], AF.Exp)

            startcol = cpool.tile([P2, 1], f32)
            nc.sync.dma_start(startcol[:], d_startcol[:])
            expstart = cpool.tile([P2, 1], f32)
            nc.scalar.activation(expstart[:], startcol[:], AF.Exp)

            cin1_dma = cpool.tile([116, T], f32)
            nc.sync.dma_start(cin1_dma[:], d_cin1[:])
            cin1 = cpool.tile([116, T], f32)
            nc.vector.tensor_copy(cin1[:], cin1_dma[:])
            ohsel_dma = cpool.tile([128, 514], f32)
            nc.sync.dma_start(ohsel_dma[:], d_ohsel[:])
            ohsel = cpool.tile([128, 514], f32)
            nc.vector.tensor_copy(ohsel[:], ohsel_dma[:])
            sel128 = cpool.tile([128, HB], f32)
            nc.sync.dma_start(sel128[:], d_sel[:])
            ident = cpool.tile([128, 128], f32)
            nc.sync.dma_start(ident[:], d_ident[:])
            maskip = cpool.tile([HB, 2052], f32)
            nc.sync.dma_start(maskip[:], d_maskip[:])
            ones102 = cpool.tile([116, 1], f32)
            nc.sync.dma_start(ones102[:], d_ones[:])

            # combo tiles: one batch-half resident at a time
            combos = {}

            def load_combo(h):
                for ch in range(NCH):
                    ct = ring.tile([128, HB, CW], bf16, tag=f"combo{ch}",
                                   bufs=1, name=f"combo{ch}")
                    nc.sync.dma_start(ct[:], d_combo[h, ch][:])
                    combos[ch] = ct

            load_combo(0)

            # ---- expl ring ----
            expl = {}

            def ensure_chunk(m):
                if m in expl or m >= NWCH:
                    return
                tl = ring.tile([P2, WCH, HB], f32, tag="explring")
                nc.sync.dma_start(tl[:], d_sct2[:, m * WCH:(m + 1) * WCH, :])
                nc.scalar.activation(tl[:], tl[:], AF.Exp)
                expl[m] = tl

            ensure_chunk(0)
            ensure_chunk(1)
            ensure_chunk(2)

            # ---- init state: exp(window0) * exp(start) ----
            # fully separate per-half state tiles keep the two chains decoupled
            state0 = spool.tile([P2, 16], f32, tag="state0")
            nc.vector.tensor_scalar_mul(state0[:], expl[0][:, 0, 0:16], expstart[:])
            state1 = spool.tile([P2, 16], f32, tag="state1")
            nc.vector.tensor_scalar_mul(state1[:], expl[0][:, 0, 16:32], expstart[:])
            states = [state0, state1]

            # staged ln(red) values: partition (w%4)*32+b', col (w//4)*2+g
            redstage = cpool.tile([128, 514], f32)
            nc.gpsimd.memset(redstage[:], 0.0)

            # ---- numerator work queue (interleaved into the loop) ----
            acc102 = cpool.tile([116, BL], f32)
            num_ops = []

            def make_num_ops():
                for h in range(2):
                    if h == 1:
                        num_ops.append(("loadh", 1))
                    for bb in range(HB):
                        b = h * HB + bb

                        def mk_mm(bb, ch):
                            def run(cd):
                                ct = combos[ch]
                                # C part (+14 zero rows): prev-onehot block
                                nc.tensor.matmul(
                                    cd[0:64, :],
                                    ct[:, bb, 102:CW],
                                    ct[:, bb, 0:T],
                                    start=(ch == 0), stop=(ch == NCH - 1),
                                    skip_group_check=True,
                                )
                                # D part: same-t products + ohlast + ohfirst
                                nc.tensor.matmul(
                                    cd[64:116, :],
                                    ct[:, bb, T:102],
                                    ct[:, bb, 0:T],
                                    start=(ch == 0), stop=(ch == NCH - 1),
                                    skip_group_check=True,
                                )
                            return run

                        def mk_ttr(b):
                            def run(cd):
                                scr = wpool.tile([116, T], f32, tag="ttr_scr",
                                                 name="ttr_scr")
                                nc.vector.scalar_tensor_tensor(
                                    acc_scr := scr[:], cd[:], 1.0, cin1[:],
                                    OP.mult, OP.mult,
                                    accum_out=acc102[:, b:b + 1],
                                )
                            return run

                        ops = [("new", b)] \
                            + [("mm", mk_mm(bb, ch)) for ch in range(NCH)] \
                            + [("ttr", mk_ttr(b))]
                        num_ops.extend(ops)

            make_num_ops()
            num_i = 0
            cur_cd = [None]

            def pump_num(k):
                nonlocal num_i
                for _ in range(k):
                    if num_i >= len(num_ops):
                        return
                    kind, payload = num_ops[num_i]
                    if kind == "new":
                        cur_cd[0] = ps_cd.tile([116, T], f32, tag="cdps", name="cdps")
                    elif kind == "loadh":
                        load_combo(1)
                    else:
                        payload(cur_cd[0])
                    num_i += 1

            # ---- recurrence ----
            def drain_red(redt, c, nw):
                # ln + transpose chunk c covering w = 16c .. 16c+nw-1
                lnc_sb = wpool.tile([2, 512], f32, tag="lnchunk", name="lnchunk")
                nc.scalar.activation(lnc_sb[:, 0:32 * nw], redt[0:2, 0:32 * nw],
                                     AF.Ln)
                for q in range((nw + 3) // 4):
                    npos = min(128, 32 * nw - 128 * q)
                    tp = ps_misc.tile([128, 2], f32, tag="misc", name="tpps")
                    nc.tensor.transpose(tp[0:npos, :],
                                        lnc_sb[:, 128 * q:128 * q + npos],
                                        ident[0:2, 0:2])
                    nc.scalar.copy(
                        redstage[0:npos, (c * 4 + q) * 2:(c * 4 + q) * 2 + 2],
                        tp[0:npos, :])

            redt = None
            for w in range(S + 1):
                if w % 16 == 0:
                    redt = ps_red.tile([2, 512], f32, tag="redps", name="redps")
                prevs = list(states)
                if w < S:
                    m = (w + 1) // WCH
                    ensure_chunk(m)
                    ensure_chunk(m + 2)
                    for ha in range(2):
                        cs = slice(16 * ha, 16 * ha + 16)
                        ps = ps_state.tile([P2, 16], f32, tag=f"stateps{ha}",
                                           name="stateps", bufs=1)
                        nc.tensor.matmul(ps[:], ehat[:, 0:P2], states[ha][:],
                                         skip_group_check=True)
                        ns = spool.tile([P2, 16], f32, tag=f"state{ha}",
                                        name="state")
                        nc.vector.scalar_tensor_tensor(
                            ns[:], ps[:], float(CINV),
                            expl[m][:, (w + 1) % WCH, cs],
                            OP.mult, OP.mult,
                        )
                        states[ha] = ns
                # red readout of current state, after the chain-critical mms
                o0 = 32 * (w % 16)
                for ha in range(2):
                    nc.tensor.matmul(redt[0:2, o0 + 16 * ha:o0 + 16 * ha + 16],
                                     ehat[:, 100:102], prevs[ha][:],
                                     skip_group_check=True)
                if w % 16 == 15:
                    drain_red(redt, w // 16, 16)
                if w == S:
                    drain_red(redt, w // 16, 1)

            pump_num(len(num_ops))

            # ---- numerator final: sum acc102 over partitions ----
            nm_ps = ps_misc.tile([BL, 1], f32, tag="misc", name="numps")
            nc.tensor.matmul(nm_ps[:], acc102[:], ones102[:], skip_group_check=True)
            num_sb = cpool.tile([BL, 1], f32)
            nc.scalar.copy(num_sb[:], nm_ps[:])
            nc.sync.dma_start(d_num[:], num_sb[:])

            # ---- denominator readout ----
            denpart = cpool.tile([128, 2], f32)
            scr_e = wpool.tile([128, 257], f32, tag="denscr", name="denscr")
            nc.vector.scalar_tensor_tensor(
                scr_e[:], redstage[:, 0:514:2], 1.0, ohsel[:, 0:514:2],
                OP.mult, OP.mult, accum_out=denpart[:, 0:1])
            scr_o = wpool.tile([128, 257], f32, tag="denscr", name="denscr")
            nc.vector.scalar_tensor_tensor(
                scr_o[:], redstage[:, 1:514:2], 1.0, ohsel[:, 1:514:2],
                OP.mult, OP.mult, accum_out=denpart[:, 1:2])
            den_ps = ps_misc.tile([HB, 2], f32, tag="misc", name="denps")
            nc.tensor.matmul(den_ps[:, 0:1], sel128[:], denpart[:, 0:1],
                             skip_group_check=True)
            nc.tensor.matmul(den_ps[:, 1:2], sel128[:], denpart[:, 1:2],
                             skip_group_check=True)
            den_sb = cpool.tile([HB, 2], f32)
            nc.scalar.copy(den_sb[:], den_ps[:])
            nc.sync.dma_start(d_den[:], den_sb[:])

            # sequence lengths per half
            li_sb = cpool.tile([HB, 2], f32)
            nc.vector.tensor_reduce(
                li_sb[:, 0:1], maskip[:, 0:2048:2], mybir.AxisListType.X, OP.add)
            nc.vector.tensor_reduce(
                li_sb[:, 1:2], maskip[:, 1:2049:2], mybir.AxisListType.X, OP.add)
            nc.sync.dma_start(d_li[:], li_sb[:])

    nc.compile()
    nc.finalize()
    return nc


def _host_inputs(token_scores, tags, token_mask, transitions,
                 start_transitions, end_transitions):
    ts = np.ascontiguousarray(token_scores, dtype=np.float32)
    tg = np.asarray(tags).astype(np.int64)
    mk = np.asarray(token_mask).astype(np.float32)
    tr = np.asarray(transitions, dtype=np.float32)
    st = np.asarray(start_transitions, dtype=np.float32)
    en = np.asarray(end_transitions, dtype=np.float32)

    # shared (replicated) constants
    ehatlog = np.full((P2, 102), NEG, np.float32)
    ehatlog[0:T, 0:T] = tr
    ehatlog[T:P2, T:P2 - 0] = tr  # cols 50:100
    ehatlog[0:T, 100] = en
    ehatlog[T:P2, 101] = en
    startcol = np.concatenate([st, st]).reshape(P2, 1).astype(np.float32)
    cin1 = np.zeros((116, T), np.float32)
    cin1[0:T] = tr
    cin1[64:114] = np.eye(T, dtype=np.float32)
    cin1[114] = en
    cin1[115] = st
    ident = np.eye(128, dtype=np.float32)
    ones102 = np.ones((116, 1), np.float32)
    sel128 = np.zeros((128, HB), np.float32)
    sel128[np.arange(128), np.arange(128) % HB] = 1.0

    ohl_full = mk - np.concatenate([mk[:, 1:], np.zeros((B, 1), np.float32)], 1)

    in_maps = []
    for r in range(NCORES):
        sl = slice(r * BL, (r + 1) * BL)
        tsc, tgc, mkc, ohlc = ts[sl], tg[sl], mk[sl], ohl_full[sl]

        sct2 = np.zeros((P2, WCH * NWCH, HB), np.float32)
        sct2[0:T, 0:S, :] = tsc[0:HB].transpose(2, 1, 0)
        sct2[T:P2, 1:S + 1, :] = tsc[HB:BL].transpose(2, 1, 0)
        sct2[T:P2, 0, :] = -st[:, None]
        # correct g1's first transition: with init v=1, (E^T v)*c must act as
        # exp(start); fold start - ln(c*colsum(E)) into the t=0 logits
        sigma = np.exp(tr.astype(np.float64)).sum(0)
        adj = (st.astype(np.float64) - np.log(np.float64(CINV) * sigma))
        sct2[T:P2, 1, :] += adj.astype(np.float32)[:, None]

        # full one-hot * mask over all t, [S, BL, T]
        oh = np.zeros((S, BL, T), np.float32)
        sidx = np.arange(S)
        bidx = np.arange(BL)
        oh[sidx[:, None], bidx[None, :], tgc[:, :].T] = 1.0
        oh *= mkc.T[:, :, None]
        ohprev = np.zeros_like(oh)
        ohprev[1:] = oh[:-1]
        combo = np.zeros((2, NCH, 128, HB, CW), np.float32)
        for h in range(2):
            bs = slice(h * HB, (h + 1) * HB)
            for ch in range(NCH):
                tt = slice(128 * ch, 128 * (ch + 1))
                combo[h, ch, :, :, 0:T] = oh[tt, bs, :]
                combo[h, ch, :, :, T:2 * T] = tsc[bs, tt, :].transpose(1, 0, 2)
                combo[h, ch, :, :, 100] = ohlc[bs, tt].T
                combo[h, ch, :, :, 102:152] = ohprev[tt, bs, :]
            combo[h, 0, 0, :, 101] = 1.0
        combo = combo.astype(ml_dtypes.bfloat16)

        maskip = np.zeros((HB, 2052), np.float32)
        maskip[:, 0:2 * S:2] = mkc[0:HB]
        maskip[:, 1:2 * S + 1:2] = mkc[HB:BL]

        # ohsel[(w%4)*32+b', (w//4)*2+g]: g=0 -> ohL[b_low, t=w] (w<=1023);
        # g=1 -> ohL[b_high, t=w-1] (w>=1)
        ohsel = np.zeros((128, 514), np.float32)
        ww = np.arange(S)
        ohsel[(ww[None, :] % 4) * 32 + np.arange(HB)[:, None],
              (ww[None, :] // 4) * 2] = ohlc[0:HB]
        wwh = np.arange(1, S + 1)
        ohsel[(wwh[None, :] % 4) * 32 + np.arange(HB)[:, None],
              (wwh[None, :] // 4) * 2 + 1] = ohlc[HB:BL]

        in_maps.append({
            "sct2": sct2,
            "combo": combo,
            "ehatlog": ehatlog,
            "startcol": startcol,
            "cin1": cin1,
            "ident": ident,
            "maskip": maskip,
            "ones102": ones102,
            "ohsel": ohsel,
            "sel128": sel128,
        })
    return in_maps


def kernel(token_scores, tags, token_mask, transitions,
           start_transitions, end_transitions):
    if "nc" not in _cached:
        _cached["nc"] = _build_nc()
    nc = _cached["nc"]

    in_maps = _host_inputs(token_scores, tags, token_mask, transitions,
                           start_transitions, end_transitions)
    res = run_bass_kernel_spmd(nc, in_maps, list(range(NCORES)), trace=TRACE)
    if TRACE and res.exec_time_ns is not None:
        _cached["exec_time_ns"] = res.exec_time_ns
        print(f"HW exec time: {res.exec_time_ns} ns")

    _cached['res'] = res
    total = np.float64(0.0)
    for r in range(NCORES):
        out = res.results[r]
        num = out["o_num"].reshape(BL)
        den = out["o_den"].reshape(HB, 2)
        li = out["o_li"].reshape(HB, 2)
        denom = den - (li - 1.0) * np.float32(LNC)
        ll = num - np.concatenate([denom[:, 0], denom[:, 1]])
        total += np.float64(ll.sum(dtype=np.float64))
    loss = -(total / B)
    return np.array(loss, dtype=np.float32)



# revision 3
# speedup vs baseline: 2.4000x; 2.4000x over previous
"""CRF loss kernel for Trainium2 (8 NeuronCores, data-parallel over batch).

Algorithm (per core, 64 sequences):
  Denominator (log-partition): exp-space forward recurrence in bf16
      s_w = (Etilde^T s_{w-1}) (*) exp(x_w)
  with an augmented 52-state vector per sequence: rows 0:50 = tag state,
  row 50 = r (red pickup), row 51 = a (accumulator). Etilde folds
  exp(transitions), exp(end) readout (col 50) and the a<-a+r carry
  (rows/cols 50:52, scaled by 82 to cancel the 1/82 step rescale that is
  folded into the emission tiles host-side). The host packs per-window
  log-multipliers x_w: live logits + ln(1/82), NEG when dead (w >= L),
  a one-shot gate on row 50 at w == L, and ln(1/82) on row 51. After S
  steps, den_raw[b] = (r + a)[b] = (1/82)^L * sum_j alpha_{L-1}[j]
  exp(end_j); the host finishes with ln(den_raw) + L*ln(82).
  Two independent 32-sequence chains ping-pong PE (matmul) and DVE
  (scalar_tensor_tensor) so neither engine idles on the serial chain.
  Numerator: per (b, t-chunk) PE matmuls against a host-packed
  [onehot(tag)*mask | scores | ohL | delta_t0 | ohprev] matrix give
  same-t products, last/first-tag one-hots and pair counts in one PSUM
  tile; one tensor_tensor_reduce against [I; end; start; trans] sums
  everything. Numerator matmuls are pumped into the recurrence loop to
  fill PE gaps.
"""

import os
import numpy as np
import ml_dtypes

import concourse.bass as bass
import concourse.bacc as bacc
import concourse.mybir as mybir
from concourse import tile
from concourse.bass_utils import run_bass_kernel_spmd

B, S, T = 512, 1024, 50
NCORES = 8
BL = B // NCORES  # 64
HB = BL // 2      # 32
P1 = T + 2        # 52: tag state + r + a rows
NEG = np.float32(-1e30)
LN82 = float(np.log(np.float64(82.0)))
LNC = -LN82

WCH = 32                       # windows per expl ring chunk
NCHK = (S + 1 + WCH - 1) // WCH  # 33 chunks cover 1025 windows
NWIN = NCHK * WCH              # 1056
NCH = 8                        # numerator chunks (128 rows each)
CW = 152                       # combo cols: oh|scores|ohL|d0|ohprev
CR = 102                       # contraction rows: eye|en|st|trans

TRACE = os.environ.get("CRF_TRACE") == "1"

_cached = {}


def _build_nc():
    f32 = mybir.dt.float32
    bf16 = mybir.dt.bfloat16
    AF = mybir.ActivationFunctionType
    OP = mybir.AluOpType

    nc = bacc.Bacc(None, target_bir_lowering=False)

    # ---- DRAM I/O ----
    d_sct2 = nc.dram_tensor("sct2", [P1, NWIN, BL], f32, kind="ExternalInput")
    d_combo = nc.dram_tensor("combo", [2, NCH, 128, HB, CW], bf16,
                             kind="ExternalInput")
    d_ebf = nc.dram_tensor("ebf", [P1, P1], bf16, kind="ExternalInput")
    d_cin1 = nc.dram_tensor("cin1", [CR, T], f32, kind="ExternalInput")
    d_ones = nc.dram_tensor("ones102", [CR, 1], f32, kind="ExternalInput")
    d_onesra = nc.dram_tensor("onesra", [P1, 1], bf16, kind="ExternalInput")

    d_num = nc.dram_tensor("o_num", [BL, 1], f32, kind="ExternalOutput")
    d_den = nc.dram_tensor("o_den", [2, HB], f32, kind="ExternalOutput")

    with tile.TileContext(nc) as tc:
        with (
            tc.tile_pool(name="const", bufs=1) as cpool,
            tc.tile_pool(name="ring", bufs=4) as ring,
            tc.tile_pool(name="state", bufs=3) as spool,
            tc.tile_pool(name="work", bufs=2) as wpool,
            tc.tile_pool(name="ps_state", bufs=2, space="PSUM") as ps_state,
            tc.tile_pool(name="ps_cd", bufs=2, space="PSUM") as ps_cd,
            tc.tile_pool(name="ps_misc", bufs=2, space="PSUM") as ps_misc,
        ):
            # ---- constants ----
            ebf_sb = cpool.tile([P1, P1], bf16)
            nc.sync.dma_start(ebf_sb[:], d_ebf[:])
            cin1 = cpool.tile([CR, T], f32)
            nc.sync.dma_start(cin1[:], d_cin1[:])
            ones102 = cpool.tile([CR, 1], f32)
            nc.sync.dma_start(ones102[:], d_ones[:])
            onesra = cpool.tile([P1, 1], bf16)
            nc.sync.dma_start(onesra[:], d_onesra[:])

            # combo tiles: all resident (fits SBUF alongside the expl ring)
            combos = {}
            for h in range(2):
                for ch in range(NCH):
                    ct = cpool.tile([128, HB, CW], bf16, tag=f"combo{h}{ch}",
                                    name=f"combo{h}{ch}")
                    nc.sync.dma_start(ct[:], d_combo[h, ch][:])
                    combos[(h, ch)] = ct

            # ---- expl ring ----
            expl = {}

            def ensure_chunk(m):
                if m in expl or m >= NCHK:
                    return
                tl = ring.tile([P1, WCH, BL], f32, tag="explring")
                nc.sync.dma_start(tl[:], d_sct2[:, m * WCH:(m + 1) * WCH, :])
                nc.scalar.activation(tl[:], tl[:], AF.Exp)
                expl[m] = tl

            ensure_chunk(0)
            ensure_chunk(1)
            ensure_chunk(2)

            # ---- init state: exp(window0) holds logits_0 + start ----
            states = []
            for ha in range(2):
                s0 = spool.tile([P1, HB], bf16, tag=f"state{ha}",
                                name="state")
                nc.scalar.copy(s0[:], expl[0][:, 0, HB * ha:HB * ha + HB])
                states.append(s0)

            # ---- numerator work queue (interleaved into the loop) ----
            acc102 = cpool.tile([CR, BL], f32)
            num_ops = []

            def make_num_ops():
                for h in range(2):
                    for bb in range(HB):
                        b = h * HB + bb

                        def mk_mm(h, bb, ch):
                            def run(cd):
                                ct = combos[(h, ch)]
                                nc.tensor.matmul(
                                    cd[:],
                                    ct[:, bb, T:CW],
                                    ct[:, bb, 0:T],
                                    start=(ch == 0), stop=(ch == NCH - 1),
                                    skip_group_check=True,
                                )
                            return run

                        def mk_ttr(b):
                            def run(cd):
                                scr = wpool.tile([CR, T], f32, tag="ttr_scr",
                                                 name="ttr_scr")
                                nc.vector.scalar_tensor_tensor(
                                    scr[:], cd[:], 1.0, cin1[:],
                                    OP.mult, OP.mult,
                                    accum_out=acc102[:, b:b + 1],
                                )
                            return run

                        ops = [("new", b)] \
                            + [("mm", mk_mm(h, bb, ch)) for ch in range(NCH)] \
                            + [("ttr", mk_ttr(b))]
                        num_ops.extend(ops)

            make_num_ops()
            num_i = 0
            cur_cd = [None]

            def pump_num(k):
                nonlocal num_i
                for _ in range(k):
                    if num_i >= len(num_ops):
                        return
                    kind, payload = num_ops[num_i]
                    if kind == "new":
                        cur_cd[0] = ps_cd.tile([CR, T], f32, tag="cdps",
                                               name="cdps")
                    else:
                        payload(cur_cd[0])
                    num_i += 1

            # ---- recurrence: w = 1..S, two chains ping-ponging ----
            PUMP_START = 16
            for w in range(1, S + 1):
                m = w // WCH
                ensure_chunk(m)
                ensure_chunk(m + 1)
                ensure_chunk(m + 2)
                for ha in range(2):
                    ps = ps_state.tile([P1, HB], f32, tag=f"ps{ha}",
                                       name="stateps", bufs=2)
                    nc.tensor.matmul(ps[:], ebf_sb[:], states[ha][:],
                                     skip_group_check=True)
                    ns = spool.tile([P1, HB], bf16, tag=f"state{ha}",
                                    name="state")
                    nc.vector.scalar_tensor_tensor(
                        ns[:], ps[:], 1.0,
                        expl[m][:, w % WCH, HB * ha:HB * ha + HB],
                        OP.mult, OP.mult,
                    )
                    states[ha] = ns
                if w >= PUMP_START:
                    k = (w * 5) // 8 - ((w - 1) * 5) // 8
                    pump_num(k)

            pump_num(len(num_ops))

            # ---- numerator final: sum acc102 over partitions ----
            nm_ps = ps_misc.tile([BL, 1], f32, tag="misc", name="numps")
            nc.tensor.matmul(nm_ps[:], acc102[:], ones102[:],
                             skip_group_check=True)
            num_sb = cpool.tile([BL, 1], f32)
            nc.scalar.copy(num_sb[:], nm_ps[:])
            nc.sync.dma_start(d_num[:], num_sb[:])

            # ---- denominator readout: den_raw = r + a per chain ----
            for ha in range(2):
                dps = ps_misc.tile([1, HB], f32, tag="misc", name="denps")
                nc.tensor.matmul(dps[:], onesra[:], states[ha][:],
                                 skip_group_check=True)
                dsb = cpool.tile([1, HB], f32)
                nc.scalar.copy(dsb[:], dps[:])
                nc.sync.dma_start(d_den[ha:ha + 1, :], dsb[:])

    nc.compile()
    nc.finalize()
    return nc


def _host_inputs(token_scores, tags, token_mask, transitions,
                 start_transitions, end_transitions):
    ts = np.ascontiguousarray(token_scores, dtype=np.float32)
    tg = np.asarray(tags).astype(np.int64)
    mk = np.asarray(token_mask).astype(np.float32)
    tr = np.asarray(transitions, dtype=np.float32)
    st = np.asarray(start_transitions, dtype=np.float32)
    en = np.asarray(end_transitions, dtype=np.float32)
    L_all = mk.sum(1).astype(np.int64)  # [B]

    # shared (replicated) constants
    ebf = np.zeros((P1, P1), np.float64)
    ebf[0:T, 0:T] = np.exp(tr.astype(np.float64))
    ebf[0:T, T] = np.exp(en.astype(np.float64))
    ebf[T, T + 1] = 82.0
    ebf[T + 1, T + 1] = 82.0
    ebf = ebf.astype(ml_dtypes.bfloat16)

    cin1 = np.zeros((CR, T), np.float32)
    cin1[0:T] = np.eye(T, dtype=np.float32)
    cin1[T] = en
    cin1[T + 1] = st
    cin1[T + 2:CR] = tr

    ones102 = np.ones((CR, 1), np.float32)
    onesra = np.zeros((P1, 1), np.float32)
    onesra[T:P1] = 1.0
    onesra = onesra.astype(ml_dtypes.bfloat16)

    ohl_full = mk - np.concatenate([mk[:, 1:], np.zeros((B, 1), np.float32)],
                                   1)

    in_maps = []
    for r in range(NCORES):
        sl = slice(r * BL, (r + 1) * BL)
        tsc, tgc, mkc, ohlc = ts[sl], tg[sl], mk[sl], ohl_full[sl]
        L = L_all[sl]

        # window tiles: [P1, NWIN, BL] of log-multipliers
        sct2 = np.full((P1, NWIN, BL), NEG, np.float32)
        logi = tsc.transpose(2, 1, 0)                       # [T, S, BL]
        live = (np.arange(S)[None, :] < L[:, None]).T       # [S, BL]
        sct2[0:T, 0:S, :] = np.where(live[None, :, :],
                                     logi + np.float32(LNC), NEG)
        sct2[0:T, 0, :] = logi[:, 0, :] + st[:, None]       # init window
        sct2[T, L, np.arange(BL)] = np.float32(LNC)         # r gate at w==L
        sct2[T + 1, 1:S + 1, :] = np.float32(LNC)           # a row

        # numerator combo, [2, NCH, 128, HB, CW]
        oh = np.zeros((S, BL, T), np.float32)
        sidx = np.arange(S)
        bidx = np.arange(BL)
        oh[sidx[:, None], bidx[None, :], tgc[:, :].T] = 1.0
        oh *= mkc.T[:, :, None]
        ohprev = np.zeros_like(oh)
        ohprev[1:] = oh[:-1]
        combo = np.zeros((2, NCH, 128, HB, CW), np.float32)
        for h in range(2):
            bs = slice(h * HB, (h + 1) * HB)
            for ch in range(NCH):
                tt = slice(128 * ch, 128 * (ch + 1))
                combo[h, ch, :, :, 0:T] = oh[tt, bs, :]
                combo[h, ch, :, :, T:2 * T] = tsc[bs, tt, :].transpose(1, 0, 2)
                combo[h, ch, :, :, 2 * T] = ohlc[bs, tt].T
                combo[h, ch, :, :, 2 * T + 2:CW] = ohprev[tt, bs, :]
            combo[h, 0, 0, :, 2 * T + 1] = 1.0
        combo = combo.astype(ml_dtypes.bfloat16)

        in_maps.append({
            "sct2": sct2,
            "combo": combo,
            "ebf": ebf,
            "cin1": cin1,
            "ones102": ones102,
            "onesra": onesra,
        })
    return in_maps, L_all


def kernel(token_scores, tags, token_mask, transitions,
           start_transitions, end_transitions):
    if "nc" not in _cached:
        _cached["nc"] = _build_nc()
    nc = _cached["nc"]

    in_maps, L_all = _host_inputs(token_scores, tags, token_mask, transitions,
                                  start_transitions, end_transitions)
    res = run_bass_kernel_spmd(nc, in_maps, list(range(NCORES)), trace=TRACE)
    if TRACE and res.exec_time_ns is not None:
        _cached["exec_time_ns"] = res.exec_time_ns
        print(f"HW exec time: {res.exec_time_ns} ns")

    _cached['res'] = res
    total = np.float64(0.0)
    for r in range(NCORES):
        out = res.results[r]
        num = out["o_num"].reshape(BL).astype(np.float64)
        den = out["o_den"].reshape(2, HB).astype(np.float64)
        denflat = np.concatenate([den[0], den[1]])
        L = L_all[r * BL:(r + 1) * BL].astype(np.float64)
        denom = np.log(denflat) + L * LN82
        ll = num - denom
        total += np.float64(ll.sum(dtype=np.float64))
    loss = -(total / B)
    return np.array(loss, dtype=np.float32)


# revision 4
# speedup vs baseline: 4.0380x; 1.6825x over previous
"""CRF loss kernel for Trainium2 (8 NeuronCores, data-parallel over batch).

Algorithm (per core, 64 sequences):
  Denominator (log-partition): exp-space recurrences in bf16, split
  meet-in-the-middle so the serial chain is S/2 long instead of S:
    forward  s_w = (Wf^T s_{w-1}) (*) exp(xf_w),   w = 1..M
    backward g_k = (Wb^T g_{k-1}) (*) exp(xb_k),   k = 1..S-1-M
  States stack two 52-row blocks (seqs 0:32 at rows 0:52, seqs 32:64 at
  rows 64:116) so one 116-wide matmul + one DVE multiply advances all 64
  sequences one step. Per 52-block: rows 0:50 = tag state, row 50/51 =
  forward r (pickup) / a (accumulate) for sequences ending at w <= M,
  with host-packed gates; the backward block uses row 50 as an exp(end)
  injection carrier that is alive for t >= L, so B_{L-1} = exp(end)
  appears exactly once. The 1/82 per-step rescale is folded into the
  emission tiles host-side; the forward r/a carry is scaled by 82 inside
  Wf to cancel it. After the loops, B_M = Wb^T g_final (one matmul) and
  den is either r+a (L <= M, scale 82^-L) or sum_j s_M[j] B_M[j]
  (L > M, scale 82^-(L-1)); both raw values go to the host, which picks
  per sequence and applies ln + L*ln(82) corrections.
  Numerator: per (b, t-chunk) PE matmuls against a host-packed
  [onehot(tag)*mask | scores | ohL | delta_t0 | ohprev] matrix give
  same-t products, last/first-tag one-hots and pair counts in one PSUM
  tile; one scalar_tensor_tensor against [I; end; start; trans] with
  accum_out sums everything. Numerator matmuls are pumped into the
  recurrence loop to fill PE gaps.
"""

import os
import numpy as np
import ml_dtypes

import concourse.bass as bass
import concourse.bacc as bacc
import concourse.mybir as mybir
from concourse import tile
from concourse.bass_utils import run_bass_kernel_spmd

B, S, T = 512, 1024, 50
NCORES = 8
BL = B // NCORES  # 64
HB = BL // 2      # 32
P1 = T + 2        # 52: tag state + 2 extra rows per block
PB = 116          # two blocks: rows 0:52 and 64:116
M = 512           # meet point: forward covers w<=M, backward t>M-? (L>M)
NEG = np.float32(-1e30)
LN82 = float(np.log(np.float64(82.0)))
LNC = -LN82

WCH = 32                 # windows per ring chunk
NCF = 17                 # forward chunks (windows 0..512 used)
NCB = 16                 # backward chunks (init + 511 steps)
NWIN = (NCF + NCB) * WCH  # 1056
NCH = 8                  # numerator chunks (128 rows each)
CW = 152                 # combo cols: oh|scores|ohL|d0|ohprev
CR = 102                 # contraction rows: eye|en|st|trans

TRACE = os.environ.get("CRF_TRACE") == "1"

_cached = {}


def _build_nc():
    f32 = mybir.dt.float32
    bf16 = mybir.dt.bfloat16
    AF = mybir.ActivationFunctionType
    OP = mybir.AluOpType

    nc = bacc.Bacc(None, target_bir_lowering=False)

    # ---- DRAM I/O ----
    d_sct = nc.dram_tensor("sct", [PB, NWIN, HB], f32, kind="ExternalInput")
    d_combo = nc.dram_tensor("combo", [2, NCH, 128, HB, CW], bf16,
                             kind="ExternalInput")
    d_ewt = nc.dram_tensor("ewt", [PB, 2 * PB], bf16, kind="ExternalInput")
    d_cin1 = nc.dram_tensor("cin1", [CR, T], f32, kind="ExternalInput")
    d_ones = nc.dram_tensor("ones102", [CR, 1], f32, kind="ExternalInput")
    d_onesl = nc.dram_tensor("onesl", [PB, 2], f32, kind="ExternalInput")
    d_onesra = nc.dram_tensor("onesra", [PB, 2], bf16, kind="ExternalInput")

    d_num = nc.dram_tensor("o_num", [BL, 1], f32, kind="ExternalOutput")
    d_den = nc.dram_tensor("o_den", [4, HB], f32, kind="ExternalOutput")

    with tile.TileContext(nc) as tc:
        with (
            tc.tile_pool(name="const", bufs=1) as cpool,
            tc.tile_pool(name="ring", bufs=3) as ring,
            tc.tile_pool(name="state", bufs=3) as spool,
            tc.tile_pool(name="work", bufs=2) as wpool,
            tc.tile_pool(name="ps_state", bufs=2, space="PSUM") as ps_state,
            tc.tile_pool(name="ps_cd", bufs=2, space="PSUM") as ps_cd,
            tc.tile_pool(name="ps_misc", bufs=2, space="PSUM") as ps_misc,
        ):
            # ---- constants ----
            ewt = cpool.tile([PB, 2 * PB], bf16)
            nc.sync.dma_start(ewt[:], d_ewt[:])
            cin1 = cpool.tile([CR, T], f32)
            nc.sync.dma_start(cin1[:], d_cin1[:])
            ones102 = cpool.tile([CR, 1], f32)
            nc.sync.dma_start(ones102[:], d_ones[:])
            onesl = cpool.tile([PB, 2], f32)
            nc.sync.dma_start(onesl[:], d_onesl[:])
            onesra = cpool.tile([PB, 2], bf16)
            nc.sync.dma_start(onesra[:], d_onesra[:])

            # combo tiles: all resident
            combos = {}
            for h in range(2):
                for ch in range(NCH):
                    ct = cpool.tile([128, HB, CW], bf16, tag=f"combo{h}{ch}",
                                    name=f"combo{h}{ch}")
                    nc.sync.dma_start(ct[:], d_combo[h, ch][:])
                    combos[(h, ch)] = ct

            # ---- expl rings (forward chunks 0..16, backward 17..32) ----
            expl = {}

            def ensure_chunk(st, m):
                c = m + (0 if st == 0 else NCF)
                if (st, m) in expl or m >= (NCF if st == 0 else NCB):
                    return
                tl = ring.tile([PB, WCH, HB], f32, tag=f"ring{st}")
                nc.sync.dma_start(tl[:], d_sct[:, c * WCH:(c + 1) * WCH, :])
                nc.scalar.activation(tl[:], tl[:], AF.Exp)
                expl[(st, m)] = tl

            for st in range(2):
                for m in range(3):
                    ensure_chunk(st, m)

            # ---- init states from window 0 of each stream ----
            states = []
            for st in range(2):
                s0 = spool.tile([PB, HB], bf16, tag=f"state{st}",
                                name="state")
                nc.scalar.copy(s0[:], expl[(st, 0)][:, 0, :])
                states.append(s0)

            # ---- numerator work queue (interleaved into the loop) ----
            acc102 = cpool.tile([CR, BL], f32)
            num_ops = []

            def make_num_ops():
                for h in range(2):
                    for bb in range(HB):
                        b = h * HB + bb

                        def mk_mm(h, bb, ch):
                            def run(cd):
                                ct = combos[(h, ch)]
                                nc.tensor.matmul(
                                    cd[:],
                                    ct[:, bb, T:CW],
                                    ct[:, bb, 0:T],
                                    start=(ch == 0), stop=(ch == NCH - 1),
                                    skip_group_check=True,
                                )
                            return run

                        def mk_ttr(b):
                            def run(cd):
                                scr = wpool.tile([CR, T], f32, tag="ttr_scr",
                                                 name="ttr_scr")
                                nc.vector.scalar_tensor_tensor(
                                    scr[:], cd[:], 1.0, cin1[:],
                                    OP.mult, OP.mult,
                                    accum_out=acc102[:, b:b + 1],
                                )
                            return run

                        ops = [("new", b)] \
                            + [("mm", mk_mm(h, bb, ch)) for ch in range(NCH)] \
                            + [("ttr", mk_ttr(b))]
                        num_ops.extend(ops)

            make_num_ops()
            num_i = 0
            cur_cd = [None]

            def pump_num(k):
                nonlocal num_i
                for _ in range(k):
                    if num_i >= len(num_ops):
                        return
                    kind, payload = num_ops[num_i]
                    if kind == "new":
                        cur_cd[0] = ps_cd.tile([CR, T], f32, tag="cdps",
                                               name="cdps")
                    else:
                        payload(cur_cd[0])
                    num_i += 1

            # ---- recurrences: fwd w = 1..M, bwd k = 1..S-1-M ----
            PUMP_START = 12
            NB = S - 1 - M  # 511 backward steps
            for w in range(1, M + 1):
                for st in range(2):
                    if st == 1 and w > NB:
                        continue
                    m = w // WCH
                    ensure_chunk(st, m)
                    ensure_chunk(st, m + 1)
                    ensure_chunk(st, m + 2)
                    ps = ps_state.tile([PB, HB], f32, tag=f"ps{st}",
                                       name="stateps", bufs=2)
                    nc.tensor.matmul(ps[:], ewt[:, st * PB:st * PB + PB],
                                     states[st][:], skip_group_check=True)
                    ns = spool.tile([PB, HB], bf16, tag=f"state{st}",
                                    name="state")
                    nc.vector.scalar_tensor_tensor(
                        ns[:], ps[:], 1.0, expl[(st, m)][:, w % WCH, :],
                        OP.mult, OP.mult,
                    )
                    states[st] = ns
                if w >= PUMP_START:
                    k = (w * 5) // 4 - ((w - 1) * 5) // 4
                    pump_num(k)

            pump_num(len(num_ops))

            # ---- numerator final: sum acc102 over partitions ----
            nm_ps = ps_misc.tile([BL, 1], f32, tag="misc", name="numps")
            nc.tensor.matmul(nm_ps[:], acc102[:], ones102[:],
                             skip_group_check=True)
            num_sb = cpool.tile([BL, 1], f32)
            nc.scalar.copy(num_sb[:], nm_ps[:])
            nc.sync.dma_start(d_num[:], num_sb[:])

            # ---- denominator meet: B_M = Wb^T g_final; meet + r/a ----
            bm_ps = ps_state.tile([PB, HB], f32, tag="ps1", name="stateps",
                                  bufs=2)
            nc.tensor.matmul(bm_ps[:], ewt[:, PB:2 * PB], states[1][:],
                             skip_group_check=True)
            prod = cpool.tile([PB, HB], f32)
            nc.vector.scalar_tensor_tensor(
                prod[:], bm_ps[:], 1.0, states[0][:], OP.mult, OP.mult)
            mt_ps = ps_misc.tile([2, HB], f32, tag="misc", name="meetps")
            nc.tensor.matmul(mt_ps[:], onesl[:], prod[:],
                             skip_group_check=True)
            mt_sb = cpool.tile([2, HB], f32)
            nc.scalar.copy(mt_sb[:], mt_ps[:])
            nc.sync.dma_start(d_den[0:2, :], mt_sb[:])
            ra_ps = ps_misc.tile([2, HB], f32, tag="misc", name="raps")
            nc.tensor.matmul(ra_ps[:], onesra[:], states[0][:],
                             skip_group_check=True)
            ra_sb = cpool.tile([2, HB], f32)
            nc.scalar.copy(ra_sb[:], ra_ps[:])
            nc.sync.dma_start(d_den[2:4, :], ra_sb[:])

    nc.compile()
    nc.finalize()
    return nc


def _host_inputs(token_scores, tags, token_mask, transitions,
                 start_transitions, end_transitions):
    ts = np.ascontiguousarray(token_scores, dtype=np.float32)
    tg = np.asarray(tags).astype(np.int64)
    mk = np.asarray(token_mask).astype(np.float32)
    tr = np.asarray(transitions, dtype=np.float32)
    st = np.asarray(start_transitions, dtype=np.float32)
    en = np.asarray(end_transitions, dtype=np.float32)
    L_all = mk.sum(1).astype(np.int64)  # [B]

    # ---- shared (replicated) constants ----
    # forward block: Wf[j,i] so out = Wf^T s; cols 0:50 tag, 50 red, 51 acc
    ef = np.zeros((P1, P1), np.float64)
    ef[0:T, 0:T] = np.exp(tr.astype(np.float64))
    ef[0:T, T] = np.exp(en.astype(np.float64))
    ef[T + 1, T + 1] = 82.0
    ef[T, T + 1] = 82.0
    # backward block: out[i] = sum_j E[i,j] g[j] + exp(end_i) carrier
    eb = np.zeros((P1, P1), np.float64)
    eb[0:T, 0:T] = np.exp(tr.astype(np.float64)).T
    eb[T, 0:T] = np.exp(en.astype(np.float64))
    eb[T, T] = 1.0

    ewt = np.zeros((PB, 2 * PB), np.float64)
    for st_i, e in ((0, ef), (1, eb)):
        ewt[0:P1, st_i * PB:st_i * PB + P1] = e
        ewt[64:64 + P1, st_i * PB + 64:st_i * PB + 64 + P1] = e
    ewt = ewt.astype(ml_dtypes.bfloat16)

    cin1 = np.zeros((CR, T), np.float32)
    cin1[0:T] = np.eye(T, dtype=np.float32)
    cin1[T] = en
    cin1[T + 1] = st
    cin1[T + 2:CR] = tr

    ones102 = np.ones((CR, 1), np.float32)
    onesl = np.zeros((PB, 2), np.float32)
    onesl[0:T, 0] = 1.0
    onesl[64:64 + T, 1] = 1.0
    onesra = np.zeros((PB, 2), np.float32)
    onesra[T:T + 2, 0] = 1.0
    onesra[64 + T:64 + T + 2, 1] = 1.0
    onesra = onesra.astype(ml_dtypes.bfloat16)

    ohl_full = mk - np.concatenate([mk[:, 1:], np.zeros((B, 1), np.float32)],
                                   1)

    in_maps = []
    for r in range(NCORES):
        sl = slice(r * BL, (r + 1) * BL)
        tsc, tgc, mkc, ohlc = ts[sl], tg[sl], mk[sl], ohl_full[sl]
        L = L_all[sl]

        # ---- window tiles: [PB, NWIN, HB] log-multipliers ----
        sct = np.full((PB, NWIN, HB), NEG, np.float32)
        for blk in range(2):
            r0 = 64 * blk
            bs = slice(blk * HB, (blk + 1) * HB)
            logi = tsc[bs].transpose(2, 1, 0)            # [T, S, HB]
            Lb = L[bs]
            live = (np.arange(S)[None, :] < Lb[:, None]).T  # [S, HB]
            # forward windows 0..M at chunk offset 0
            sct[r0:r0 + T, 0:M + 1, :] = np.where(
                live[None, 0:M + 1, :], logi[:, 0:M + 1, :] + np.float32(LNC),
                NEG)
            sct[r0:r0 + T, 0, :] = logi[:, 0, :] + st[:, None]
            gl = np.minimum(Lb, M)                        # gate at w == L
            sct[r0 + T, gl, np.arange(HB)] = np.where(Lb <= M,
                                                      np.float32(LNC), NEG)
            sct[r0 + T + 1, 1:M + 1, :] = np.float32(LNC)
            # backward: window NCF*WCH + k holds x for t = S - k; k=0 init
            boff = NCF * WCH
            sct[r0 + T, boff, :] = 0.0                    # carrier init
            kk = np.arange(1, S - M)                      # 1..511
            tt = S - kk
            livb = (tt[None, :] < Lb[:, None]).T          # [NB, HB]
            sct[r0:r0 + T, boff + 1:boff + S - M, :] = np.where(
                livb[None, :, :],
                logi[:, tt, :] + np.float32(LNC), NEG)
            sct[r0 + T, boff + 1:boff + S - M, :] = np.where(
                (tt[None, :] >= Lb[:, None]).T, np.float32(0.0), NEG)

        # ---- numerator combo, [2, NCH, 128, HB, CW] ----
        oh = np.zeros((S, BL, T), np.float32)
        sidx = np.arange(S)
        bidx = np.arange(BL)
        oh[sidx[:, None], bidx[None, :], tgc[:, :].T] = 1.0
        oh *= mkc.T[:, :, None]
        ohprev = np.zeros_like(oh)
        ohprev[1:] = oh[:-1]
        combo = np.zeros((2, NCH, 128, HB, CW), np.float32)
        for h in range(2):
            bs = slice(h * HB, (h + 1) * HB)
            for ch in range(NCH):
                tt2 = slice(128 * ch, 128 * (ch + 1))
                combo[h, ch, :, :, 0:T] = oh[tt2, bs, :]
                combo[h, ch, :, :, T:2 * T] = \
                    tsc[bs, tt2, :].transpose(1, 0, 2)
                combo[h, ch, :, :, 2 * T] = ohlc[bs, tt2].T
                combo[h, ch, :, :, 2 * T + 2:CW] = ohprev[tt2, bs, :]
            combo[h, 0, 0, :, 2 * T + 1] = 1.0
        combo = combo.astype(ml_dtypes.bfloat16)

        in_maps.append({
            "sct": sct,
            "combo": combo,
            "ewt": ewt,
            "cin1": cin1,
            "ones102": ones102,
            "onesl": onesl,
            "onesra": onesra,
        })
    return in_maps, L_all


def kernel(token_scores, tags, token_mask, transitions,
           start_transitions, end_transitions):
    if "nc" not in _cached:
        _cached["nc"] = _build_nc()
    nc = _cached["nc"]

    in_maps, L_all = _host_inputs(token_scores, tags, token_mask, transitions,
                                  start_transitions, end_transitions)
    res = run_bass_kernel_spmd(nc, in_maps, list(range(NCORES)), trace=TRACE)
    if TRACE and res.exec_time_ns is not None:
        _cached["exec_time_ns"] = res.exec_time_ns
        print(f"HW exec time: {res.exec_time_ns} ns")

    _cached['res'] = res
    total = np.float64(0.0)
    for r in range(NCORES):
        out = res.results[r]
        num = out["o_num"].reshape(BL).astype(np.float64)
        den = out["o_den"].reshape(4, HB).astype(np.float64)
        meet = np.concatenate([den[0], den[1]])
        ra = np.concatenate([den[2], den[3]])
        L = L_all[r * BL:(r + 1) * BL]
        Lf = L.astype(np.float64)
        with np.errstate(divide="ignore"):
            denom = np.where(L <= M,
                             np.log(ra) + Lf * LN82,
                             np.log(meet) + (Lf - 1.0) * LN82)
        ll = num - denom
        total += np.float64(ll.sum(dtype=np.float64))
    loss = -(total / B)
    return np.array(loss, dtype=np.float32)
